# revision 37
# baseline (speedup 1.0000x reference)
"""Trainium2 Bass kernel for nn_BiholoModelFuncGENERALforHYMinv3.

Computation (per sample):
  x[18] -> 9 complex coords in 3 projective factors of 3
  bihom feature chain -> sec[729] (divided by kappa product)
  two towers: u1=(sec@W1+b1)^2 -> u2=(.@W2+b2)^2 -> u3=(.@W3+b3)^2
  out = Wfa*log(u3a) - Wfb*log(u3b), clipped to +-1e6

Distribution: pure data parallel over batch, 8 NeuronCores, 4096 samples
per core. Weights replicated.

On-chip layout: features/hidden units on the partition axis, batch on the
free axis. The bihom chain runs on DVE with batch on partitions via
broadcast-AP outer products up to the level-2 vector L2[81] and the
kappa-folded factor-2 vector f2[9]; those are PE-transposed, replicated
into chunk-aligned PSUM by constant selection matmuls, and one
elementwise mul per 128-row chunk forms featsT = L2[i]*f2[j] directly in
feature-on-partition order.

Precision plan (rel-err budget 2e-2; this config measures ~7e-3 in the
numpy pipeline model):
  - L1 and L2 run as fp8e4m3 DoubleRow matmuls: 2 fp8 weights per PE
    cell -> K=256 contraction per instruction at the same ~N-cycle
    streaming cost, i.e. 2x MAC throughput vs fp32r (HW-measured 217 ns
    pitch at N=512 vs 313 ns for fp32r K=128).
  - Power-of-2 scales keep operands in e4m3 range (TRN e4m3 infinity at
    240): feats*64 (folded into the SELF selection constants), W1*128 /
    W2*128 (host-side), q1*512 (folded into the L1 ACT square's
    scale/bias: out = (sqrt(512)/(64*128) * psum + sqrt(512)*b1)^2).
    The L2 ACT divides everything back out (scale 2^-16) so q2 is
    unscaled.
  - q2 / W3 stay high precision (bf16): the final contraction feeds
    log(z3^2) whose error the log amplifies; fp8 there alone measures
    2e-2. bf16 matmuls cost the same per instruction as fp32r but with
    hidden (4x faster) weight loads.
  - The final W3 contraction is accumulated inside the L2 loop with a
    one-chunk lag so the in-order PE never waits on the ACT square it
    just issued.
"""
import numpy as np

N_CORES = 8
B_FULL = 32768
B_CORE = B_FULL // N_CORES
N_TILE = 512          # moving-dim per tower pass (fp8 DR rhs hits the 1024 cap)
H = 1024              # hidden width
NSEC = 729
SEC_CHUNKS = [128, 128, 128, 128, 128, 89]   # 729 = 5*128 + 89

S_F = 64.0            # feats fp8 scale (folded into SELF selection consts)
S_W1 = 128.0          # W1 fp8 scale (host-side)
S_Q1 = 512.0          # q1 fp8 scale (folded into L1 ACT scale/bias)
ACT1_SCALE = float(np.sqrt(S_Q1) / (S_F * S_W1))


def _brd(t_ap, free_dims, import_bass):
    """AP with t_ap's partition dim plus custom free [step,count] dims."""
    bass = import_bass
    return bass.AP(tensor=t_ap.tensor, offset=t_ap.offset,
                   ap=[list(t_ap.ap[0])] + [list(d) for d in free_dims])


def build_nc(ACT2S, b_core=B_CORE, n_tile=N_TILE, finalize=True):
    """ACT2S[t]: the L2 ACT scale 1/(s_q1*s_w2) undoing the fp8 operand
    scaling (s_w2 is chosen per tower host-side)."""
    import concourse.bass as bass
    import concourse.tile as tile
    from concourse import mybir, bacc
    from concourse.masks import make_identity

    F32 = mybir.dt.float32
    F32R = mybir.dt.float32r
    F8 = mybir.dt.float8e4
    BF16 = mybir.dt.bfloat16
    AF = mybir.ActivationFunctionType
    ALU = mybir.AluOpType
    DR = mybir.MatmulPerfMode.DoubleRow

    assert b_core % n_tile == 0 and n_tile % 128 == 0
    n_macro = b_core // n_tile
    n_sub = n_tile // 128

    nc = bacc.Bacc()
    n_sub_total = b_core // 128
    x_d = nc.declare_dram_parameter("x", [128, n_sub_total * 18], F32, isOutput=False)
    wd = {}
    for t in ("a", "b"):
        wd["W1" + t] = nc.declare_dram_parameter("W1" + t, [768, H], F8, isOutput=False)
        wd["b1" + t] = nc.declare_dram_parameter("b1" + t, [H], F32, isOutput=False)
        wd["W2" + t] = nc.declare_dram_parameter("W2" + t, [H, H], F8, isOutput=False)
        wd["b2" + t] = nc.declare_dram_parameter("b2" + t, [H], F32, isOutput=False)
        wd["W3" + t] = nc.declare_dram_parameter("W3" + t, [H, 1], BF16, isOutput=False)
        wd["b3" + t] = nc.declare_dram_parameter("b3" + t, [1], F32, isOutput=False)
        wd["Wf" + t] = nc.declare_dram_parameter("Wf" + t, [1, 1], F32, isOutput=False)
    out_d = nc.declare_dram_parameter("out", [b_core], F32, isOutput=True)

    with tile.TileContext(nc) as tc:
        import contextlib
        with contextlib.ExitStack() as ctx:
            consts = ctx.enter_context(tc.tile_pool(name="consts", bufs=1))
            xp = ctx.enter_context(tc.tile_pool(name="xp", bufs=4))
            ft = ctx.enter_context(tc.tile_pool(name="ft", bufs=2))
            ftp = ctx.enter_context(tc.tile_pool(name="ftp", bufs=2))
            lftp = ctx.enter_context(tc.tile_pool(name="lftp", bufs=1))
            qp = ctx.enter_context(tc.tile_pool(name="qp", bufs=2))
            q2p = ctx.enter_context(tc.tile_pool(name="q2p", bufs=2))
            ep = ctx.enter_context(tc.tile_pool(name="ep", bufs=1))
            psT = ctx.enter_context(tc.tile_pool(name="psT", bufs=1, space="PSUM"))
            psL = ctx.enter_context(tc.tile_pool(name="psL", bufs=3, space="PSUM"))
            psU = ctx.enter_context(tc.tile_pool(name="psU", bufs=2, space="PSUM"))
            self2 = ctx.enter_context(tc.tile_pool(name="self2", bufs=1, space="PSUM"))

            # ---- constants / weights (resident) ----
            ident = consts.tile([128, 128], BF16, tag="ident", name="ident")
            make_identity(nc, ident[:])
            # Selection matrices for featsT assembly: featsT global row
            # g = j*81 + i (j: factor-2 idx, i: level-2 idx). For chunk c,
            # SELL[:, c, :].T @ L2T replicates L2T row i(g) into partition
            # g%128, and SELF[:, c, :].T @ f2T9 replicates f2 row j(g).
            # SELF carries S_F so featsT comes out pre-scaled for fp8.
            selp_cm = tc.tile_pool(name="selp", bufs=1)
            selp = selp_cm.__enter__()
            SELL = consts.tile([81, 6, 128], BF16, tag="SELL", name="SELL")
            SELF_ = consts.tile([9, 6, 128], BF16, tag="SELF", name="SELF")
            for c in range(6):
                SELL0 = selp.tile([81, 128], F32, tag="SELL0", name="SELL0")
                nc.gpsimd.memset(SELL0[:], 0.0)
                s_c = (128 * c) % 81
                for w in range(3):
                    d = s_c - 81 * w
                    if not (-127 <= d <= 80):
                        continue
                    nc.gpsimd.affine_select(
                        out=SELL0[:], in_=SELL0[:],
                        compare_op=mybir.AluOpType.not_equal, fill=1.0,
                        base=-d, pattern=[[-1, 128]], channel_multiplier=1)
                nc.scalar.activation(out=SELL[:, c, :], in_=SELL0[:],
                                     func=AF.Identity)
                SELF0 = selp.tile([9, 128], F32, tag="SELL0", name="SELF0")
                nc.gpsimd.memset(SELF0[:], S_F)
                # keep S_F only where 0 <= 128c + p - 81j <= 80
                nc.gpsimd.affine_select(
                    out=SELF0[:], in_=SELF0[:],
                    compare_op=mybir.AluOpType.is_ge, fill=0.0,
                    base=128 * c, pattern=[[1, 128]], channel_multiplier=-81)
                nc.gpsimd.affine_select(
                    out=SELF0[:], in_=SELF0[:],
                    compare_op=mybir.AluOpType.is_ge, fill=0.0,
                    base=80 - 128 * c, pattern=[[-1, 128]], channel_multiplier=81)
                nc.scalar.activation(out=SELF_[:, c, :], in_=SELF0[:],
                                     func=AF.Identity)
            selp_cm.__exit__(None, None, None)

            # x lands per-macro so macro 0's features start after ~300KB,
            # not after the whole 2.25MB image
            x_sb = consts.tile([128, n_sub_total * 18], F32, tag="x_sb", name="x_sb")
            for mt in range(n_macro):
                c0, c1 = mt * n_sub * 18, (mt + 1) * n_sub * 18
                nc.sync.dma_start(out=x_sb[:, c0:c1], in_=x_d[:, c0:c1])

            W1 = {}; W2 = {}; W3 = {}; B1 = {}; B2 = {}; B3 = {}; WF = {}
            scal4 = consts.tile([1, 4], F32, tag="scal4", name="scal4")
            B3["a"] = scal4[0:1, 0:1]; B3["b"] = scal4[0:1, 1:2]
            WF["a"] = scal4[0:1, 2:3]; WF["b"] = scal4[0:1, 3:4]
            for t in ("a", "b"):
                W1[t] = consts.tile([128, 6, H], F8, tag="W1" + t, name="W1" + t)
                W2[t] = consts.tile([128, 8, H], F8, tag="W2" + t, name="W2" + t)
                W3[t] = consts.tile([128, 8, 1], BF16, tag="W3" + t, name="W3" + t)
                B1[t] = consts.tile([128, 8], F32, tag="b1" + t, name="b1" + t)
                B2[t] = consts.tile([128, 8], F32, tag="b2" + t, name="b2" + t)

            # One DMA per weight tile (region-aligned so each consumer
            # matmul waits on exactly one queue); all tensors start
            # streaming concurrently on different queues; issue order
            # matches first-use order in the tower loop.
            for t in ("a", "b"):
                nc.sync.dma_start(out=W1[t][:],
                                  in_=wd["W1" + t][:, :].rearrange("(k p) h -> p k h", p=128))
                # W2 in halves so macro 0's L2 starts on the first half
                nc.sync.dma_start(out=W2[t][:, 0:4, :],
                                  in_=wd["W2" + t][0:512, :].rearrange("(k p) h -> p k h", p=128))
                nc.sync.dma_start(out=W2[t][:, 4:8, :],
                                  in_=wd["W2" + t][512:1024, :].rearrange("(k p) h -> p k h", p=128))
            for t in ("a", "b"):
                nc.sync.dma_start(out=W3[t][:],
                                  in_=wd["W3" + t][:, :].rearrange("(k p) h -> p k h", p=128))
                nc.sync.dma_start(out=B1[t][:], in_=wd["b1" + t].rearrange("(m p) -> p m", p=128))
                nc.sync.dma_start(out=B2[t][:], in_=wd["b2" + t].rearrange("(m p) -> p m", p=128))
                nc.sync.dma_start(out=B3[t], in_=wd["b3" + t].rearrange("(p o) -> p o", o=1))
                nc.sync.dma_start(out=WF[t], in_=wd["Wf" + t][:, :])

            def feats_subtile(x_t, eng):
                """Compute the level-2/factor-2 vectors for one 128-sample
                subtile. x_t: [128, 18] slice (batch on partitions).
                eng: nc.vector or nc.gpsimd -- whole chain runs there
                (except reciprocal), so alternating subtiles pipeline on
                two engines."""
                xr = x_t[:, 0:9]
                xi = x_t[:, 9:18]

                # full 3x3 grids for all 3 factors: [128, 27], idx f*9+a*3+b
                XX = ft.tile([128, 27], F32, tag="XX", name="XX")
                eng.tensor_mul(XX[:], _brd(xr, [[3, 3], [1, 3], [0, 3]], bass),
                               _brd(xr, [[3, 3], [0, 3], [1, 3]], bass))
                XXYY = ft.tile([128, 27], F32, tag="XXYY", name="XXYY")
                eng.tensor_mul(XXYY[:], _brd(xi, [[3, 3], [1, 3], [0, 3]], bass),
                               _brd(xi, [[3, 3], [0, 3], [1, 3]], bass))
                eng.tensor_add(XXYY[:], XXYY[:], XX[:])
                # kappa [128, 3] = diag sums; kprod, inv (recip early: it
                # is DVE-only, so GpSimd subtiles need the round trip)
                kap = ft.tile([128, 3], F32, tag="kap", name="kap")
                eng.tensor_add(kap[:], _brd(XXYY[:, 0:1], [[9, 3]], bass),
                               _brd(XXYY[:, 4:5], [[9, 3]], bass))
                eng.tensor_add(kap[:], kap[:], _brd(XXYY[:, 8:9], [[9, 3]], bass))
                kp = ft.tile([128, 1], F32, tag="kp", name="kp")
                eng.tensor_mul(kp[:], kap[:, 0:1], kap[:, 1:2])
                eng.tensor_mul(kp[:], kp[:], kap[:, 2:3])
                inv = ft.tile([128, 1], F32, tag="inv", name="inv")
                nc.vector.reciprocal(inv[:], kp[:])
                XY = ft.tile([128, 27], F32, tag="XY", name="XY")
                eng.tensor_mul(XY[:], _brd(xr, [[3, 3], [1, 3], [0, 3]], bass),
                               _brd(xi, [[3, 3], [0, 3], [1, 3]], bass))

                # r_all [128, 3, 6]: triu-gather cols {0,1,2,4,5,8} of each grid
                r_all = ft.tile([128, 3, 6], F32, tag="r_all", name="r_all")
                eng.tensor_copy(_brd(r_all[:, 0, 0:3], [[6, 3], [1, 3]], bass),
                                _brd(XXYY[:, 0:3], [[9, 3], [1, 3]], bass))
                eng.tensor_copy(_brd(r_all[:, 0, 3:5], [[6, 3], [1, 2]], bass),
                                _brd(XXYY[:, 4:6], [[9, 3], [1, 2]], bass))
                eng.tensor_copy(_brd(r_all[:, 0, 5:6], [[6, 3], [1, 1]], bass),
                                _brd(XXYY[:, 8:9], [[9, 3], [1, 1]], bass))
                # im_all [128, 3, 3]: XY[a,b]-XY[b,a] for (0,1),(0,2),(1,2)
                # (im1's sign for the L2 blocks is folded into W1 host-side)
                im_all = ft.tile([128, 3, 3], F32, tag="im_all", name="im_all")
                eng.tensor_sub(_brd(im_all[:, 0, 0:2], [[3, 3], [1, 2]], bass),
                               _brd(XY[:, 1:3], [[9, 3], [1, 2]], bass),
                               _brd(XY[:, 3:7], [[9, 3], [3, 2]], bass))
                eng.tensor_sub(_brd(im_all[:, 0, 2:3], [[3, 3], [1, 1]], bass),
                               _brd(XY[:, 5:6], [[9, 3], [1, 1]], bass),
                               _brd(XY[:, 7:8], [[9, 3], [1, 1]], bass))
                r0 = r_all[:, 0, :]
                r1 = r_all[:, 1, :]
                i0 = im_all[:, 0, :]
                i1 = im_all[:, 1, :]
                # LF [128, 90]: level-2 vector L2 (81 = [R2 45 | I2 36]) and
                # kappa-folded factor-2 vector f2 (9 = [rr2 6 | ii2 3]),
                # batch on partitions. One PE transpose turns it into
                # [90, 128] feature-on-partition.
                LF = ft.tile([128, 90], BF16, tag="LF", name="LF")
                eng.tensor_mul(LF[:, 0:36], _brd(r0, [[1, 6], [0, 6]], bass),
                               _brd(r1, [[0, 6], [1, 6]], bass))
                eng.tensor_mul(LF[:, 36:45], _brd(i0, [[1, 3], [0, 3]], bass),
                               _brd(i1[:, 0:3], [[0, 3], [1, 3]], bass))
                eng.tensor_mul(LF[:, 45:63], _brd(r0, [[1, 6], [0, 3]], bass),
                               _brd(i1[:, 0:3], [[0, 6], [1, 3]], bass))
                eng.tensor_mul(LF[:, 63:81], _brd(i0, [[1, 3], [0, 6]], bass),
                               _brd(r1, [[0, 3], [1, 6]], bass))
                eng.tensor_mul(LF[:, 81:87], r_all[:, 2, :],
                               _brd(inv[:, 0:1], [[0, 6]], bass))
                eng.tensor_mul(LF[:, 87:90], im_all[:, 2, :],
                               _brd(inv[:, 0:1], [[0, 3]], bass))
                return LF

            def feats_T(mt):
                """DVE features + PE transposes/selection-matmuls ->
                featsT [128, 6, n_tile] (fp8e4, feature-on-partition,
                pre-scaled by S_F)."""
                featsT = ftp.tile([128, 6, n_tile], F8, tag="featsT", name="featsT")
                # zero the 768-pad rows of chunk 5 (DoubleRow partner of 4);
                # engines need a 32-aligned base partition, so clear 64:128
                # and let the chunk-5 mul below rewrite 64:89 (WAW-ordered)
                nc.gpsimd.memset(featsT[64:128, 5, :], 0.0)
                L2T = lftp.tile([81, n_tile], BF16, tag="L2T", name="L2T")
                f2T9 = lftp.tile([9, n_tile], BF16, tag="f2T9", name="f2T9")
                for s in range(n_sub):
                    gs = mt * n_sub + s
                    LF = feats_subtile(x_sb[:, gs * 18:gs * 18 + 18], nc.vector)
                    pT = psT.tile([128, 256], BF16, tag="pT", name="pT")
                    nc.tensor.transpose(pT[0:81, 0:128], LF[:, 0:81], ident[:])
                    nc.tensor.transpose(pT[0:9, 128:256], LF[:, 81:90], ident[:])
                    nc.vector.tensor_copy(L2T[:, s * 128:(s + 1) * 128], pT[0:81, 0:128])
                    nc.vector.tensor_copy(f2T9[:, s * 128:(s + 1) * 128], pT[0:9, 128:256])
                for c in range(6):
                    kk = SEC_CHUNKS[c]
                    psL2 = self2.tile([128, n_tile], F32, tag="psL2", name="psL2")
                    nc.tensor.matmul(psL2[:], SELL[:, c, :], L2T[:],
                                     start=True, stop=True)
                    psF2 = self2.tile([128, n_tile], F32, tag="psF2", name="psF2")
                    nc.tensor.matmul(psF2[:], SELF_[:, c, :], f2T9[:],
                                     start=True, stop=True)
                    f2S = lftp.tile([128, n_tile], F32, tag="f2S", name="f2S",
                                    bufs=1)
                    nc.vector.tensor_copy(f2S[:kk, :], psF2[:kk, :])
                    nc.vector.tensor_mul(featsT[:kk, c, :], psL2[:kk, :], f2S[:kk, :])
                return featsT

            def layer1(featsT, t):
                """L1 for one tower: fp8 DoubleRow over 3 K=256 chunk pairs."""
                q1 = qp.tile([128, 8, n_tile], F8, tag="q1" + t, name="q1" + t,
                             bufs=(2 if t == "a" else 1))
                for m in range(8):
                    ps = psL.tile([128, n_tile], F32, tag="mm", name="mm")
                    for c in range(3):
                        nc.tensor.matmul(ps[:],
                                         W1[t][:, 2 * c:2 * c + 2, m * 128:(m + 1) * 128],
                                         featsT[:, 2 * c:2 * c + 2, :],
                                         start=(c == 0), stop=(c == 2),
                                         perf_mode=DR)
                    nc.scalar.activation(out=q1[:, m, :], in_=ps[:], func=AF.Square,
                                         bias=B1[t][:, m:m + 1], scale=ACT1_SCALE)
                return q1

            def layer23(mt, q1s):
                """L2 (fp8 DR) with fused bf16 L3 accumulation + epilogue."""
                base = mt * n_tile
                lns = {}
                for t in ("a", "b"):
                    q1 = q1s[t]
                    u3 = psU.tile([1, n_tile], F32, tag="u3", name="u3")
                    q2_pend = []
                    for m in range(8):
                        ps = psL.tile([128, n_tile], F32, tag="mm", name="mm")
                        for c in range(4):
                            nc.tensor.matmul(ps[:],
                                             W2[t][:, 2 * c:2 * c + 2, m * 128:(m + 1) * 128],
                                             q1[:, 2 * c:2 * c + 2, :],
                                             start=(c == 0), stop=(c == 3),
                                             perf_mode=DR)
                        q2m = q2p.tile([128, n_tile], BF16, tag="q2m", name="q2m")
                        nc.scalar.activation(out=q2m[:], in_=ps[:], func=AF.Square,
                                             bias=B2[t][:, m:m + 1], scale=ACT2S[t])
                        # lag the u3 accumulation one m-chunk so the in-order
                        # PE never waits on the ACT square it just requested
                        q2_pend.append((m, q2m))
                        if len(q2_pend) == 2:
                            mm_, q2m_ = q2_pend.pop(0)
                            nc.tensor.matmul(u3[:], W3[t][:, mm_, :], q2m_[:],
                                             start=(mm_ == 0), stop=False)
                    mm_, q2m_ = q2_pend.pop(0)
                    nc.tensor.matmul(u3[:], W3[t][:, mm_, :], q2m_[:],
                                     start=False, stop=True)
                    ln_t = ep.tile([1, n_tile], F32, tag="ln" + t, name="ln" + t)
                    nc.scalar.activation(out=ln_t[:], in_=u3[:], func=AF.Square,
                                         bias=B3[t], scale=1.0)
                    nc.scalar.activation(out=ln_t[:], in_=ln_t[:], func=AF.Ln)
                    lns[t] = ln_t
                nc.vector.tensor_scalar_mul(lns["b"][:], lns["b"][:], WF["b"])
                nc.vector.scalar_tensor_tensor(out=lns["a"][:], in0=lns["a"][:],
                                               scalar=WF["a"], in1=lns["b"][:],
                                               op0=ALU.mult, op1=ALU.subtract)
                nc.vector.tensor_scalar(out=lns["a"][:], in0=lns["a"][:], scalar1=1.0e6,
                                        scalar2=-1.0e6, op0=ALU.min, op1=ALU.max)
                nc.sync.dma_start(out=out_d[base:base + n_tile], in_=lns["a"][:])

            # Software-pipelined emission: features + tower-a L1 run one
            # macro ahead (they only need x and W1a, so they fill the PE
            # while the W2/W3 DMAs land); tower-b L1 is emitted just before
            # its consumer to keep q1b at one buffer.
            fT = {0: feats_T(0)}
            q1a_pre = {}
            for mt in range(n_macro):
                if mt + 1 < n_macro:
                    fT[mt + 1] = feats_T(mt + 1)
                    if mt == 0:
                        # startup-only: macro 1's tower-a L1 needs just W1a,
                        # giving the PE work while the W2/W3 DMAs land
                        q1a_pre[1] = layer1(fT[1], "a")
                cur = fT.pop(mt)
                q1a = q1a_pre.pop(mt) if mt in q1a_pre else layer1(cur, "a")
                q1b = layer1(cur, "b")
                layer23(mt, {"a": q1a, "b": q1b})

    if finalize:
        nc.finalize()   # Bacc pass pipeline: reg alloc, wait splitting, etc.
    return nc


def _w1_perm():
    """featsT row g = j*81 + i holds L2vec[i]*f2vec[j]; reference sec index
    for that product (blocks: R2xrr2 | I2nxii2 | R2xii2 | I2nxrr2)."""
    perm = np.empty(NSEC, np.int64)
    for j in range(9):
        for i in range(81):
            if i < 45:
                ref = i * 6 + j if j < 6 else 378 + i * 3 + (j - 6)
            else:
                ii = i - 45
                ref = 513 + ii * 6 + j if j < 6 else 270 + ii * 3 + (j - 6)
            perm[j * 81 + i] = ref
    return perm


def _to_e4m3(w, scale):
    """Scale, clip to TRN e4m3 range, quantize (RNE)."""
    import ml_dtypes
    return np.clip(w * scale, -240.0, 240.0).astype(ml_dtypes.float8_e4m3)


def prep_weights(inputs):
    """Per-core weight dict + per-tower L2 ACT scale.

    W1: rows permuted to the kernel's featsT order, padded to 768, scaled
    e4m3. W2: scaled e4m3 (per-tower pow2 scale). W3: bf16. b1 pre-scaled
    by sqrt(S_Q1) to match the L1 ACT square."""
    import ml_dtypes
    inp = {k: np.asarray(v, dtype=np.float32) for k, v in inputs.items()
           if k != "x"}
    perm = _w1_perm()
    weights = {}
    act2s = {}
    # the kernel computes L2[36:63] with +im1 (no negation op); the
    # reference wants -im1 there, so those W1 rows flip sign
    i_idx = np.arange(768) % 81
    sgn = np.where((i_idx >= 36) & (i_idx < 63), -1.0, 1.0).astype(np.float32)
    for t in ("a", "b"):
        w1 = np.zeros((768, H), np.float32)
        w1[:NSEC] = inp["W1" + t][perm]
        w1 *= sgn[:, None]
        weights["W1" + t] = np.ascontiguousarray(_to_e4m3(w1, S_W1))
        weights["b1" + t] = (inp["b1" + t] * np.sqrt(S_Q1)).astype(np.float32)
        w2 = inp["W2" + t]
        s_w2 = float(2.0 ** np.floor(np.log2(96.0 / max(np.abs(w2).max(), 1e-30))))
        weights["W2" + t] = np.ascontiguousarray(_to_e4m3(w2, s_w2))
        act2s[t] = 1.0 / (S_Q1 * s_w2)
        weights["b2" + t] = inp["b2" + t]
        weights["W3" + t] = inp["W3" + t].astype(ml_dtypes.bfloat16)
        weights["b3" + t] = inp["b3" + t]
        weights["Wf" + t] = inp["Wf" + t]
    return weights, act2s


def prep_x(x_core):
    """Per-core x [b, 18] -> SBUF image [128, (b/128)*18]: partition p holds
    sample p of each 128-row subtile, subtiles concatenated along free dim."""
    b = x_core.shape[0]
    n_sub_total = b // 128
    return np.ascontiguousarray(
        x_core.reshape(n_sub_total, 128, 18).transpose(1, 0, 2).reshape(128, n_sub_total * 18))


def run(inputs, trace=False, b_core=B_CORE, n_tile=N_TILE, n_cores=N_CORES):
    """Shard inputs, run the SPMD kernel on n_cores, gather full output.
    Returns (out [B,1] fp32, BassKernelResults)."""
    from concourse import bass_utils

    weights, act2s = prep_weights(inputs)
    nc = build_nc(act2s, b_core=b_core, n_tile=n_tile)
    x = np.ascontiguousarray(np.asarray(inputs["x"], dtype=np.float32))
    in_maps = []
    for c in range(n_cores):
        m = {"x": prep_x(x[c * b_core:(c + 1) * b_core])}
        m.update(weights)
        in_maps.append(m)
    res = bass_utils.run_bass_kernel_spmd(nc, in_maps, core_ids=list(range(n_cores)),
                                          trace=trace)
    out = np.concatenate([r["out"] for r in res.results], axis=0)
    return out.reshape(-1, 1).astype(np.float32), res


def kernel(**inputs) -> np.ndarray:
    out, _ = run(inputs, trace=False)
    return out


# revision 38
# speedup vs baseline: 1.0016x; 1.0016x over previous
"""Trainium2 Bass kernel for nn_BiholoModelFuncGENERALforHYMinv3.

Computation (per sample):
  x[18] -> 9 complex coords in 3 projective factors of 3
  bihom feature chain -> sec[729] (divided by kappa product)
  two towers: u1=(sec@W1+b1)^2 -> u2=(.@W2+b2)^2 -> u3=(.@W3+b3)^2
  out = Wfa*log(u3a) - Wfb*log(u3b), clipped to +-1e6

Distribution: pure data parallel over batch, 8 NeuronCores, 4096 samples
per core. Weights replicated.

On-chip layout: features/hidden units on the partition axis, batch on the
free axis. The bihom chain runs on DVE with batch on partitions via
broadcast-AP outer products up to the level-2 vector L2[81] and the
kappa-folded factor-2 vector f2[9]; those are PE-transposed, replicated
into chunk-aligned PSUM by constant selection matmuls, and one
elementwise mul per 128-row chunk forms featsT = L2[i]*f2[j] directly in
feature-on-partition order.

Precision plan (rel-err budget 2e-2; this config measures ~7e-3 in the
numpy pipeline model):
  - L1 and L2 run as fp8e4m3 DoubleRow matmuls: 2 fp8 weights per PE
    cell -> K=256 contraction per instruction at the same ~N-cycle
    streaming cost, i.e. 2x MAC throughput vs fp32r (HW-measured 217 ns
    pitch at N=512 vs 313 ns for fp32r K=128).
  - Power-of-2 scales keep operands in e4m3 range (TRN e4m3 infinity at
    240): feats*64 (folded into the SELF selection constants), W1*128 /
    W2*128 (host-side), q1*512 (folded into the L1 ACT square's
    scale/bias: out = (sqrt(512)/(64*128) * psum + sqrt(512)*b1)^2).
    The L2 ACT divides everything back out (scale 2^-16) so q2 is
    unscaled.
  - q2 / W3 stay high precision (bf16): the final contraction feeds
    log(z3^2) whose error the log amplifies; fp8 there alone measures
    2e-2. bf16 matmuls cost the same per instruction as fp32r but with
    hidden (4x faster) weight loads.
  - The final W3 contraction is accumulated inside the L2 loop with a
    one-chunk lag so the in-order PE never waits on the ACT square it
    just issued.
"""
import numpy as np

N_CORES = 8
B_FULL = 32768
B_CORE = B_FULL // N_CORES
N_TILE = 512          # moving-dim per tower pass (fp8 DR rhs hits the 1024 cap)
H = 1024              # hidden width
NSEC = 729
SEC_CHUNKS = [128, 128, 128, 128, 128, 89]   # 729 = 5*128 + 89

S_F = 64.0            # feats fp8 scale (folded into SELF selection consts)
S_W1 = 128.0          # W1 fp8 scale (host-side)
S_Q1 = 512.0          # q1 fp8 scale (folded into L1 ACT scale/bias)
ACT1_SCALE = float(np.sqrt(S_Q1) / (S_F * S_W1))


def _brd(t_ap, free_dims, import_bass):
    """AP with t_ap's partition dim plus custom free [step,count] dims."""
    bass = import_bass
    return bass.AP(tensor=t_ap.tensor, offset=t_ap.offset,
                   ap=[list(t_ap.ap[0])] + [list(d) for d in free_dims])


def build_nc(ACT2S, b_core=B_CORE, n_tile=N_TILE, finalize=True):
    """ACT2S[t]: the L2 ACT scale 1/(s_q1*s_w2) undoing the fp8 operand
    scaling (s_w2 is chosen per tower host-side)."""
    import concourse.bass as bass
    import concourse.tile as tile
    from concourse import mybir, bacc
    from concourse.masks import make_identity

    F32 = mybir.dt.float32
    F32R = mybir.dt.float32r
    F8 = mybir.dt.float8e4
    BF16 = mybir.dt.bfloat16
    AF = mybir.ActivationFunctionType
    ALU = mybir.AluOpType
    DR = mybir.MatmulPerfMode.DoubleRow

    assert b_core % n_tile == 0 and n_tile % 128 == 0
    n_macro = b_core // n_tile
    n_sub = n_tile // 128

    nc = bacc.Bacc()
    n_sub_total = b_core // 128
    x_d = nc.declare_dram_parameter("x", [128, n_sub_total * 18], F32, isOutput=False)
    wd = {}
    for t in ("a", "b"):
        wd["W1" + t] = nc.declare_dram_parameter("W1" + t, [768, H], F8, isOutput=False)
        wd["b1" + t] = nc.declare_dram_parameter("b1" + t, [H], F32, isOutput=False)
        wd["W2" + t] = nc.declare_dram_parameter("W2" + t, [H, H], F8, isOutput=False)
        wd["b2" + t] = nc.declare_dram_parameter("b2" + t, [H], F32, isOutput=False)
        wd["W3" + t] = nc.declare_dram_parameter("W3" + t, [H, 1], BF16, isOutput=False)
        wd["b3" + t] = nc.declare_dram_parameter("b3" + t, [1], F32, isOutput=False)
        wd["Wf" + t] = nc.declare_dram_parameter("Wf" + t, [1, 1], F32, isOutput=False)
    out_d = nc.declare_dram_parameter("out", [b_core], F32, isOutput=True)

    with tile.TileContext(nc) as tc:
        import contextlib
        with contextlib.ExitStack() as ctx:
            consts = ctx.enter_context(tc.tile_pool(name="consts", bufs=1))
            xp = ctx.enter_context(tc.tile_pool(name="xp", bufs=4))
            ft = ctx.enter_context(tc.tile_pool(name="ft", bufs=2))
            ftp = ctx.enter_context(tc.tile_pool(name="ftp", bufs=2))
            lftp = ctx.enter_context(tc.tile_pool(name="lftp", bufs=1))
            qp = ctx.enter_context(tc.tile_pool(name="qp", bufs=2))
            q2p = ctx.enter_context(tc.tile_pool(name="q2p", bufs=2))
            ep = ctx.enter_context(tc.tile_pool(name="ep", bufs=1))
            psT = ctx.enter_context(tc.tile_pool(name="psT", bufs=1, space="PSUM"))
            psL = ctx.enter_context(tc.tile_pool(name="psL", bufs=3, space="PSUM"))
            psU = ctx.enter_context(tc.tile_pool(name="psU", bufs=2, space="PSUM"))
            self2 = ctx.enter_context(tc.tile_pool(name="self2", bufs=1, space="PSUM"))

            # ---- constants / weights (resident) ----
            ident = consts.tile([128, 128], BF16, tag="ident", name="ident")
            make_identity(nc, ident[:])
            # Selection matrices for featsT assembly: featsT global row
            # g = j*81 + i (j: factor-2 idx, i: level-2 idx). For chunk c,
            # SELL[:, c, :].T @ L2T replicates L2T row i(g) into partition
            # g%128, and SELF[:, c, :].T @ f2T9 replicates f2 row j(g).
            # SELF carries S_F so featsT comes out pre-scaled for fp8.
            selp_cm = tc.tile_pool(name="selp", bufs=1)
            selp = selp_cm.__enter__()
            SELL = consts.tile([81, 6, 128], BF16, tag="SELL", name="SELL")
            SELF_ = consts.tile([9, 6, 128], BF16, tag="SELF", name="SELF")
            for c in range(6):
                SELL0 = selp.tile([81, 128], F32, tag="SELL0", name="SELL0")
                nc.gpsimd.memset(SELL0[:], 0.0)
                s_c = (128 * c) % 81
                for w in range(3):
                    d = s_c - 81 * w
                    if not (-127 <= d <= 80):
                        continue
                    nc.gpsimd.affine_select(
                        out=SELL0[:], in_=SELL0[:],
                        compare_op=mybir.AluOpType.not_equal, fill=1.0,
                        base=-d, pattern=[[-1, 128]], channel_multiplier=1)
                nc.scalar.activation(out=SELL[:, c, :], in_=SELL0[:],
                                     func=AF.Identity)
                SELF0 = selp.tile([9, 128], F32, tag="SELL0", name="SELF0")
                nc.gpsimd.memset(SELF0[:], S_F)
                # keep S_F only where 0 <= 128c + p - 81j <= 80
                nc.gpsimd.affine_select(
                    out=SELF0[:], in_=SELF0[:],
                    compare_op=mybir.AluOpType.is_ge, fill=0.0,
                    base=128 * c, pattern=[[1, 128]], channel_multiplier=-81)
                nc.gpsimd.affine_select(
                    out=SELF0[:], in_=SELF0[:],
                    compare_op=mybir.AluOpType.is_ge, fill=0.0,
                    base=80 - 128 * c, pattern=[[-1, 128]], channel_multiplier=81)
                nc.scalar.activation(out=SELF_[:, c, :], in_=SELF0[:],
                                     func=AF.Identity)
            selp_cm.__exit__(None, None, None)

            # x lands per-macro so macro 0's features start after ~300KB,
            # not after the whole 2.25MB image
            x_sb = consts.tile([128, n_sub_total * 18], F32, tag="x_sb", name="x_sb")
            for mt in range(n_macro):
                c0, c1 = mt * n_sub * 18, (mt + 1) * n_sub * 18
                nc.sync.dma_start(out=x_sb[:, c0:c1], in_=x_d[:, c0:c1])

            W1 = {}; W2 = {}; W3 = {}; B1 = {}; B2 = {}; B3 = {}; WF = {}
            scal4 = consts.tile([1, 4], F32, tag="scal4", name="scal4")
            B3["a"] = scal4[0:1, 0:1]; B3["b"] = scal4[0:1, 1:2]
            WF["a"] = scal4[0:1, 2:3]; WF["b"] = scal4[0:1, 3:4]
            for t in ("a", "b"):
                W1[t] = consts.tile([128, 6, H], F8, tag="W1" + t, name="W1" + t)
                W2[t] = consts.tile([128, 8, H], F8, tag="W2" + t, name="W2" + t)
                W3[t] = consts.tile([128, 8, 1], BF16, tag="W3" + t, name="W3" + t)
                B1[t] = consts.tile([128, 8], F32, tag="b1" + t, name="b1" + t)
                B2[t] = consts.tile([128, 8], F32, tag="b2" + t, name="b2" + t)

            # One DMA per weight tile (region-aligned so each consumer
            # matmul waits on exactly one queue); all tensors start
            # streaming concurrently on different queues; issue order
            # matches first-use order in the tower loop.
            for t in ("a", "b"):
                nc.sync.dma_start(out=W1[t][:],
                                  in_=wd["W1" + t][:, :].rearrange("(k p) h -> p k h", p=128))
                # W2 in halves so macro 0's L2 starts on the first half
                nc.sync.dma_start(out=W2[t][:, 0:4, :],
                                  in_=wd["W2" + t][0:512, :].rearrange("(k p) h -> p k h", p=128))
                nc.sync.dma_start(out=W2[t][:, 4:8, :],
                                  in_=wd["W2" + t][512:1024, :].rearrange("(k p) h -> p k h", p=128))
            for t in ("a", "b"):
                nc.sync.dma_start(out=W3[t][:],
                                  in_=wd["W3" + t][:, :].rearrange("(k p) h -> p k h", p=128))
                nc.sync.dma_start(out=B1[t][:], in_=wd["b1" + t].rearrange("(m p) -> p m", p=128))
                nc.sync.dma_start(out=B2[t][:], in_=wd["b2" + t].rearrange("(m p) -> p m", p=128))
                nc.sync.dma_start(out=B3[t], in_=wd["b3" + t].rearrange("(p o) -> p o", o=1))
                nc.sync.dma_start(out=WF[t], in_=wd["Wf" + t][:, :])

            def feats_subtile(x_t, eng):
                """Compute the level-2/factor-2 vectors for one 128-sample
                subtile. x_t: [128, 18] slice (batch on partitions).
                eng: nc.vector or nc.gpsimd -- whole chain runs there
                (except reciprocal), so alternating subtiles pipeline on
                two engines."""
                xr = x_t[:, 0:9]
                xi = x_t[:, 9:18]

                # full 3x3 grids for all 3 factors: [128, 27], idx f*9+a*3+b
                XX = ft.tile([128, 27], F32, tag="XX", name="XX")
                eng.tensor_mul(XX[:], _brd(xr, [[3, 3], [1, 3], [0, 3]], bass),
                               _brd(xr, [[3, 3], [0, 3], [1, 3]], bass))
                XXYY = ft.tile([128, 27], F32, tag="XXYY", name="XXYY")
                eng.tensor_mul(XXYY[:], _brd(xi, [[3, 3], [1, 3], [0, 3]], bass),
                               _brd(xi, [[3, 3], [0, 3], [1, 3]], bass))
                eng.tensor_add(XXYY[:], XXYY[:], XX[:])
                # kappa [128, 3] = diag sums; kprod, inv (recip early: it
                # is DVE-only, so GpSimd subtiles need the round trip)
                kap = ft.tile([128, 3], F32, tag="kap", name="kap")
                eng.tensor_add(kap[:], _brd(XXYY[:, 0:1], [[9, 3]], bass),
                               _brd(XXYY[:, 4:5], [[9, 3]], bass))
                eng.tensor_add(kap[:], kap[:], _brd(XXYY[:, 8:9], [[9, 3]], bass))
                kp = ft.tile([128, 1], F32, tag="kp", name="kp")
                eng.tensor_mul(kp[:], kap[:, 0:1], kap[:, 1:2])
                eng.tensor_mul(kp[:], kp[:], kap[:, 2:3])
                inv = ft.tile([128, 1], F32, tag="inv", name="inv")
                nc.vector.reciprocal(inv[:], kp[:])
                XY = ft.tile([128, 27], F32, tag="XY", name="XY")
                eng.tensor_mul(XY[:], _brd(xr, [[3, 3], [1, 3], [0, 3]], bass),
                               _brd(xi, [[3, 3], [0, 3], [1, 3]], bass))

                # r_all [128, 3, 6]: triu-gather cols {0,1,2,4,5,8} of each grid
                r_all = ft.tile([128, 3, 6], F32, tag="r_all", name="r_all")
                eng.tensor_copy(_brd(r_all[:, 0, 0:3], [[6, 3], [1, 3]], bass),
                                _brd(XXYY[:, 0:3], [[9, 3], [1, 3]], bass))
                eng.tensor_copy(_brd(r_all[:, 0, 3:5], [[6, 3], [1, 2]], bass),
                                _brd(XXYY[:, 4:6], [[9, 3], [1, 2]], bass))
                eng.tensor_copy(_brd(r_all[:, 0, 5:6], [[6, 3], [1, 1]], bass),
                                _brd(XXYY[:, 8:9], [[9, 3], [1, 1]], bass))
                # im_all [128, 3, 3]: XY[a,b]-XY[b,a] for (0,1),(0,2),(1,2)
                # (im1's sign for the L2 blocks is folded into W1 host-side)
                im_all = ft.tile([128, 3, 3], F32, tag="im_all", name="im_all")
                eng.tensor_sub(_brd(im_all[:, 0, 0:2], [[3, 3], [1, 2]], bass),
                               _brd(XY[:, 1:3], [[9, 3], [1, 2]], bass),
                               _brd(XY[:, 3:7], [[9, 3], [3, 2]], bass))
                eng.tensor_sub(_brd(im_all[:, 0, 2:3], [[3, 3], [1, 1]], bass),
                               _brd(XY[:, 5:6], [[9, 3], [1, 1]], bass),
                               _brd(XY[:, 7:8], [[9, 3], [1, 1]], bass))
                r0 = r_all[:, 0, :]
                r1 = r_all[:, 1, :]
                i0 = im_all[:, 0, :]
                i1 = im_all[:, 1, :]
                # LF [128, 90]: level-2 vector L2 (81 = [R2 45 | I2 36]) and
                # kappa-folded factor-2 vector f2 (9 = [rr2 6 | ii2 3]),
                # batch on partitions. One PE transpose turns it into
                # [90, 128] feature-on-partition.
                LF = ft.tile([128, 90], BF16, tag="LF", name="LF")
                eng.tensor_mul(LF[:, 0:36], _brd(r0, [[1, 6], [0, 6]], bass),
                               _brd(r1, [[0, 6], [1, 6]], bass))
                eng.tensor_mul(LF[:, 36:45], _brd(i0, [[1, 3], [0, 3]], bass),
                               _brd(i1[:, 0:3], [[0, 3], [1, 3]], bass))
                eng.tensor_mul(LF[:, 45:63], _brd(r0, [[1, 6], [0, 3]], bass),
                               _brd(i1[:, 0:3], [[0, 6], [1, 3]], bass))
                eng.tensor_mul(LF[:, 63:81], _brd(i0, [[1, 3], [0, 6]], bass),
                               _brd(r1, [[0, 3], [1, 6]], bass))
                eng.tensor_mul(LF[:, 81:87], r_all[:, 2, :],
                               _brd(inv[:, 0:1], [[0, 6]], bass))
                eng.tensor_mul(LF[:, 87:90], im_all[:, 2, :],
                               _brd(inv[:, 0:1], [[0, 3]], bass))
                return LF

            def feats_T(mt):
                """DVE features + PE transposes/selection-matmuls ->
                featsT [128, 6, n_tile] (fp8e4, feature-on-partition,
                pre-scaled by S_F)."""
                featsT = ftp.tile([128, 6, n_tile], F8, tag="featsT", name="featsT")
                # zero the 768-pad rows of chunk 5 (DoubleRow partner of 4);
                # engines need a 32-aligned base partition, so clear 64:128
                # and let the chunk-5 mul below rewrite 64:89 (WAW-ordered)
                nc.gpsimd.memset(featsT[64:128, 5, :], 0.0)
                L2T = lftp.tile([81, n_tile], BF16, tag="L2T", name="L2T")
                f2T9 = lftp.tile([9, n_tile], BF16, tag="f2T9", name="f2T9")
                for s in range(n_sub):
                    gs = mt * n_sub + s
                    LF = feats_subtile(x_sb[:, gs * 18:gs * 18 + 18], nc.vector)
                    pT = psT.tile([128, 256], BF16, tag="pT", name="pT")
                    nc.tensor.transpose(pT[0:81, 0:128], LF[:, 0:81], ident[:])
                    nc.tensor.transpose(pT[0:9, 128:256], LF[:, 81:90], ident[:])
                    nc.vector.tensor_copy(L2T[:, s * 128:(s + 1) * 128], pT[0:81, 0:128])
                    nc.vector.tensor_copy(f2T9[:, s * 128:(s + 1) * 128], pT[0:9, 128:256])
                for c in range(6):
                    kk = SEC_CHUNKS[c]
                    psL2 = self2.tile([128, n_tile], F32, tag="psL2", name="psL2")
                    nc.tensor.matmul(psL2[:], SELL[:, c, :], L2T[:],
                                     start=True, stop=True)
                    psF2 = self2.tile([128, n_tile], F32, tag="psF2", name="psF2")
                    nc.tensor.matmul(psF2[:], SELF_[:, c, :], f2T9[:],
                                     start=True, stop=True)
                    f2S = lftp.tile([128, n_tile], F32, tag="f2S", name="f2S",
                                    bufs=1)
                    nc.vector.tensor_copy(f2S[:kk, :], psF2[:kk, :])
                    nc.vector.tensor_mul(featsT[:kk, c, :], psL2[:kk, :], f2S[:kk, :])
                return featsT

            def layer1(featsT, t):
                """L1 for one tower: fp8 DoubleRow over 3 K=256 chunk pairs."""
                q1 = qp.tile([128, 8, n_tile], F8, tag="q1" + t, name="q1" + t,
                             bufs=2)
                for m in range(8):
                    ps = psL.tile([128, n_tile], F32, tag="mm", name="mm")
                    for c in range(3):
                        nc.tensor.matmul(ps[:],
                                         W1[t][:, 2 * c:2 * c + 2, m * 128:(m + 1) * 128],
                                         featsT[:, 2 * c:2 * c + 2, :],
                                         start=(c == 0), stop=(c == 2),
                                         perf_mode=DR)
                    nc.scalar.activation(out=q1[:, m, :], in_=ps[:], func=AF.Square,
                                         bias=B1[t][:, m:m + 1], scale=ACT1_SCALE)
                return q1

            def layer23(mt, q1s):
                """L2 (fp8 DR) with fused bf16 L3 accumulation + epilogue."""
                base = mt * n_tile
                lns = {}
                for t in ("a", "b"):
                    q1 = q1s[t]
                    u3 = psU.tile([1, n_tile], F32, tag="u3", name="u3")
                    q2_pend = []
                    for m in range(8):
                        ps = psL.tile([128, n_tile], F32, tag="mm", name="mm")
                        for c in range(4):
                            nc.tensor.matmul(ps[:],
                                             W2[t][:, 2 * c:2 * c + 2, m * 128:(m + 1) * 128],
                                             q1[:, 2 * c:2 * c + 2, :],
                                             start=(c == 0), stop=(c == 3),
                                             perf_mode=DR)
                        q2m = q2p.tile([128, n_tile], BF16, tag="q2m", name="q2m")
                        nc.scalar.activation(out=q2m[:], in_=ps[:], func=AF.Square,
                                             bias=B2[t][:, m:m + 1], scale=ACT2S[t])
                        # lag the u3 accumulation one m-chunk so the in-order
                        # PE never waits on the ACT square it just requested
                        q2_pend.append((m, q2m))
                        if len(q2_pend) == 2:
                            mm_, q2m_ = q2_pend.pop(0)
                            nc.tensor.matmul(u3[:], W3[t][:, mm_, :], q2m_[:],
                                             start=(mm_ == 0), stop=False)
                    mm_, q2m_ = q2_pend.pop(0)
                    nc.tensor.matmul(u3[:], W3[t][:, mm_, :], q2m_[:],
                                     start=False, stop=True)
                    ln_t = ep.tile([1, n_tile], F32, tag="ln" + t, name="ln" + t)
                    nc.scalar.activation(out=ln_t[:], in_=u3[:], func=AF.Square,
                                         bias=B3[t], scale=1.0)
                    nc.scalar.activation(out=ln_t[:], in_=ln_t[:], func=AF.Ln)
                    lns[t] = ln_t
                nc.vector.tensor_scalar_mul(lns["b"][:], lns["b"][:], WF["b"])
                nc.vector.scalar_tensor_tensor(out=lns["a"][:], in0=lns["a"][:],
                                               scalar=WF["a"], in1=lns["b"][:],
                                               op0=ALU.mult, op1=ALU.subtract)
                nc.vector.tensor_scalar(out=lns["a"][:], in0=lns["a"][:], scalar1=1.0e6,
                                        scalar2=-1.0e6, op0=ALU.min, op1=ALU.max)
                nc.sync.dma_start(out=out_d[base:base + n_tile], in_=lns["a"][:])

            # Software-pipelined emission: features + tower-a L1 run one
            # macro ahead (they only need x and W1a, so they fill the PE
            # while the W2/W3 DMAs land); tower-b L1 is emitted just before
            # its consumer to keep q1b at one buffer.
            fT = {0: feats_T(0)}
            q1a_pre = {}
            for mt in range(n_macro):
                if mt + 1 < n_macro:
                    fT[mt + 1] = feats_T(mt + 1)
                    if mt == 0:
                        # startup-only: macro 1's tower-a L1 needs just W1a,
                        # giving the PE work while the W2/W3 DMAs land
                        q1a_pre[1] = layer1(fT[1], "a")
                cur = fT.pop(mt)
                q1a = q1a_pre.pop(mt) if mt in q1a_pre else layer1(cur, "a")
                q1b = layer1(cur, "b")
                layer23(mt, {"a": q1a, "b": q1b})

    if finalize:
        nc.finalize()   # Bacc pass pipeline: reg alloc, wait splitting, etc.
    return nc


def _w1_perm():
    """featsT row g = j*81 + i holds L2vec[i]*f2vec[j]; reference sec index
    for that product (blocks: R2xrr2 | I2nxii2 | R2xii2 | I2nxrr2)."""
    perm = np.empty(NSEC, np.int64)
    for j in range(9):
        for i in range(81):
            if i < 45:
                ref = i * 6 + j if j < 6 else 378 + i * 3 + (j - 6)
            else:
                ii = i - 45
                ref = 513 + ii * 6 + j if j < 6 else 270 + ii * 3 + (j - 6)
            perm[j * 81 + i] = ref
    return perm


def _to_e4m3(w, scale):
    """Scale, clip to TRN e4m3 range, quantize (RNE)."""
    import ml_dtypes
    return np.clip(w * scale, -240.0, 240.0).astype(ml_dtypes.float8_e4m3)


def prep_weights(inputs):
    """Per-core weight dict + per-tower L2 ACT scale.

    W1: rows permuted to the kernel's featsT order, padded to 768, scaled
    e4m3. W2: scaled e4m3 (per-tower pow2 scale). W3: bf16. b1 pre-scaled
    by sqrt(S_Q1) to match the L1 ACT square."""
    import ml_dtypes
    inp = {k: np.asarray(v, dtype=np.float32) for k, v in inputs.items()
           if k != "x"}
    perm = _w1_perm()
    weights = {}
    act2s = {}
    # the kernel computes L2[36:63] with +im1 (no negation op); the
    # reference wants -im1 there, so those W1 rows flip sign
    i_idx = np.arange(768) % 81
    sgn = np.where((i_idx >= 36) & (i_idx < 63), -1.0, 1.0).astype(np.float32)
    for t in ("a", "b"):
        w1 = np.zeros((768, H), np.float32)
        w1[:NSEC] = inp["W1" + t][perm]
        w1 *= sgn[:, None]
        weights["W1" + t] = np.ascontiguousarray(_to_e4m3(w1, S_W1))
        weights["b1" + t] = (inp["b1" + t] * np.sqrt(S_Q1)).astype(np.float32)
        w2 = inp["W2" + t]
        s_w2 = float(2.0 ** np.floor(np.log2(96.0 / max(np.abs(w2).max(), 1e-30))))
        weights["W2" + t] = np.ascontiguousarray(_to_e4m3(w2, s_w2))
        act2s[t] = 1.0 / (S_Q1 * s_w2)
        weights["b2" + t] = inp["b2" + t]
        weights["W3" + t] = inp["W3" + t].astype(ml_dtypes.bfloat16)
        weights["b3" + t] = inp["b3" + t]
        weights["Wf" + t] = inp["Wf" + t]
    return weights, act2s


def prep_x(x_core):
    """Per-core x [b, 18] -> SBUF image [128, (b/128)*18]: partition p holds
    sample p of each 128-row subtile, subtiles concatenated along free dim."""
    b = x_core.shape[0]
    n_sub_total = b // 128
    return np.ascontiguousarray(
        x_core.reshape(n_sub_total, 128, 18).transpose(1, 0, 2).reshape(128, n_sub_total * 18))


def run(inputs, trace=False, b_core=B_CORE, n_tile=N_TILE, n_cores=N_CORES):
    """Shard inputs, run the SPMD kernel on n_cores, gather full output.
    Returns (out [B,1] fp32, BassKernelResults)."""
    from concourse import bass_utils

    weights, act2s = prep_weights(inputs)
    nc = build_nc(act2s, b_core=b_core, n_tile=n_tile)
    x = np.ascontiguousarray(np.asarray(inputs["x"], dtype=np.float32))
    in_maps = []
    for c in range(n_cores):
        m = {"x": prep_x(x[c * b_core:(c + 1) * b_core])}
        m.update(weights)
        in_maps.append(m)
    res = bass_utils.run_bass_kernel_spmd(nc, in_maps, core_ids=list(range(n_cores)),
                                          trace=trace)
    out = np.concatenate([r["out"] for r in res.results], axis=0)
    return out.reshape(-1, 1).astype(np.float32), res


def kernel(**inputs) -> np.ndarray:
    out, _ = run(inputs, trace=False)
    return out


# revision 39
# speedup vs baseline: 1.2001x; 1.1981x over previous
"""Trainium2 Bass kernel for nn_BiholoModelFuncGENERALforHYMinv3.

Computation (per sample):
  x[18] -> 9 complex coords in 3 projective factors of 3
  bihom feature chain -> sec[729] (divided by kappa product)
  two towers: u1=(sec@W1+b1)^2 -> u2=(.@W2+b2)^2 -> u3=(.@W3+b3)^2
  out = Wfa*log(u3a) - Wfb*log(u3b), clipped to +-1e6

Distribution: pure data parallel over batch, 8 NeuronCores, 4096 samples
per core. Weights replicated.

On-chip layout: features/hidden units on the partition axis, batch on the
free axis. The bihom chain runs on DVE with batch on partitions via
broadcast-AP outer products up to the level-2 vector L2[81] and the
kappa-folded factor-2 vector f2[9] (bf16); those are PE-transposed,
replicated into chunk-aligned PSUM by constant bf16 selection matmuls,
and one elementwise mul per 128-row chunk forms featsT = L2[i]*f2[j]
directly in feature-on-partition order (the im1 negation the reference
uses in two L2 blocks is folded into W1 row signs host-side).

Precision plan (rel-err budget 2e-2; measures 7.3e-3 on HW, matching the
numpy pipeline model):
  - L1 and L2 run as fp8e4m3 DoubleRow matmuls: 2 fp8 weights per PE
    cell -> K=256 contraction per instruction at the same ~N-cycle
    streaming cost, i.e. 2x MAC throughput vs fp32r (HW-measured 217 ns
    pitch at N=512 vs 313 ns for fp32r K=128). W1 is padded 729->768 so
    its 6 chunks pair into 3 K=256 groups (pad rows exactly zero).
  - Power-of-2 scales keep operands in e4m3 range (TRN e4m3 infinity at
    240): feats*64 (folded into the SELF selection constants), W1*128,
    W2*pow2 per tower (host-side), q1*512 (folded into the L1 ACT
    square's scale/bias). The L2 ACT divides everything back out so q2
    is unscaled.
  - q2 / W3 stay high precision (bf16): the final contraction feeds
    log(z3^2) whose error the log amplifies; fp8 there alone measures
    2e-2. bf16 matmuls cost the same per instruction as fp32r but with
    hidden (4x faster) weight loads.
  - The final W3 contraction is accumulated inside the L2 loop with a
    one-chunk lag so the in-order PE never waits on the ACT square it
    just issued.

Schedule: macro tiles of 512 samples; features for macro mt+1 (DVE) are
emitted before macro mt's towers so they overlap on different engines;
macro 1's tower-a L1 is additionally hoisted to cover the W2 DMA landing
at startup. x and W2 land in per-macro / half-tensor DMA pieces.

Note: the shared TRN2 devices flip between a 2.4 GHz and a ~2.0 GHz PE
clock state (P0 power-state downclock); wall-clock for the same NEFF
varies ~20% run to run (333 us fast-state, ~398 us slow-state).
"""
import numpy as np

N_CORES = 8
B_FULL = 32768
B_CORE = B_FULL // N_CORES
N_TILE = 512          # moving-dim per tower pass (fp8 DR rhs hits the 1024 cap)
H = 1024              # hidden width
NSEC = 729
SEC_CHUNKS = [128, 128, 128, 128, 128, 89]   # 729 = 5*128 + 89

S_F = 64.0            # feats fp8 scale (folded into SELF selection consts)
S_W1 = 128.0          # W1 fp8 scale (host-side)
S_Q1 = 512.0          # q1 fp8 scale (folded into L1 ACT scale/bias)
ACT1_SCALE = float(np.sqrt(S_Q1) / (S_F * S_W1))


def _brd(t_ap, free_dims, import_bass):
    """AP with t_ap's partition dim plus custom free [step,count] dims."""
    bass = import_bass
    return bass.AP(tensor=t_ap.tensor, offset=t_ap.offset,
                   ap=[list(t_ap.ap[0])] + [list(d) for d in free_dims])


def build_nc(ACT2S, b_core=B_CORE, n_tile=N_TILE, finalize=True):
    """ACT2S[t]: the L2 ACT scale 1/(s_q1*s_w2) undoing the fp8 operand
    scaling (s_w2 is chosen per tower host-side)."""
    import concourse.bass as bass
    import concourse.tile as tile
    from concourse import mybir, bacc
    from concourse.masks import make_identity

    F32 = mybir.dt.float32
    F32R = mybir.dt.float32r
    F8 = mybir.dt.float8e4
    BF16 = mybir.dt.bfloat16
    AF = mybir.ActivationFunctionType
    ALU = mybir.AluOpType
    DR = mybir.MatmulPerfMode.DoubleRow

    assert b_core % n_tile == 0 and n_tile % 128 == 0
    n_macro = b_core // n_tile
    n_sub = n_tile // 128

    nc = bacc.Bacc()
    n_sub_total = b_core // 128
    x_d = nc.declare_dram_parameter("x", [128, n_sub_total * 18], F32, isOutput=False)
    wd = {}
    for t in ("a", "b"):
        wd["W1" + t] = nc.declare_dram_parameter("W1" + t, [768, H], F8, isOutput=False)
        wd["b1" + t] = nc.declare_dram_parameter("b1" + t, [H], F32, isOutput=False)
        wd["W2" + t] = nc.declare_dram_parameter("W2" + t, [H, H], F8, isOutput=False)
        wd["b2" + t] = nc.declare_dram_parameter("b2" + t, [H], F32, isOutput=False)
        wd["W3" + t] = nc.declare_dram_parameter("W3" + t, [H, 1], BF16, isOutput=False)
        wd["b3" + t] = nc.declare_dram_parameter("b3" + t, [1], F32, isOutput=False)
        wd["Wf" + t] = nc.declare_dram_parameter("Wf" + t, [1, 1], F32, isOutput=False)
    out_d = nc.declare_dram_parameter("out", [b_core], F32, isOutput=True)

    with tile.TileContext(nc) as tc:
        import contextlib
        with contextlib.ExitStack() as ctx:
            consts = ctx.enter_context(tc.tile_pool(name="consts", bufs=1))
            xp = ctx.enter_context(tc.tile_pool(name="xp", bufs=4))
            ft = ctx.enter_context(tc.tile_pool(name="ft", bufs=2))
            ftp = ctx.enter_context(tc.tile_pool(name="ftp", bufs=2))
            lftp = ctx.enter_context(tc.tile_pool(name="lftp", bufs=1))
            qp = ctx.enter_context(tc.tile_pool(name="qp", bufs=2))
            q2p = ctx.enter_context(tc.tile_pool(name="q2p", bufs=2))
            ep = ctx.enter_context(tc.tile_pool(name="ep", bufs=1))
            psT = ctx.enter_context(tc.tile_pool(name="psT", bufs=1, space="PSUM"))
            psL = ctx.enter_context(tc.tile_pool(name="psL", bufs=3, space="PSUM"))
            psU = ctx.enter_context(tc.tile_pool(name="psU", bufs=2, space="PSUM"))
            self2 = ctx.enter_context(tc.tile_pool(name="self2", bufs=1, space="PSUM"))

            # ---- constants / weights (resident) ----
            ident = consts.tile([128, 128], BF16, tag="ident", name="ident")
            make_identity(nc, ident[:])
            # Selection matrices for featsT assembly: featsT global row
            # g = j*81 + i (j: factor-2 idx, i: level-2 idx). For chunk c,
            # SELL[:, c, :].T @ L2T replicates L2T row i(g) into partition
            # g%128, and SELF[:, c, :].T @ f2T9 replicates f2 row j(g).
            # SELF carries S_F so featsT comes out pre-scaled for fp8.
            selp_cm = tc.tile_pool(name="selp", bufs=1)
            selp = selp_cm.__enter__()
            SELL = consts.tile([81, 6, 128], BF16, tag="SELL", name="SELL")
            SELF_ = consts.tile([9, 6, 128], BF16, tag="SELF", name="SELF")
            for c in range(6):
                SELL0 = selp.tile([81, 128], F32, tag="SELL0", name="SELL0")
                nc.gpsimd.memset(SELL0[:], 0.0)
                s_c = (128 * c) % 81
                for w in range(3):
                    d = s_c - 81 * w
                    if not (-127 <= d <= 80):
                        continue
                    nc.gpsimd.affine_select(
                        out=SELL0[:], in_=SELL0[:],
                        compare_op=mybir.AluOpType.not_equal, fill=1.0,
                        base=-d, pattern=[[-1, 128]], channel_multiplier=1)
                nc.scalar.activation(out=SELL[:, c, :], in_=SELL0[:],
                                     func=AF.Identity)
                SELF0 = selp.tile([9, 128], F32, tag="SELL0", name="SELF0")
                nc.gpsimd.memset(SELF0[:], S_F)
                # keep S_F only where 0 <= 128c + p - 81j <= 80
                nc.gpsimd.affine_select(
                    out=SELF0[:], in_=SELF0[:],
                    compare_op=mybir.AluOpType.is_ge, fill=0.0,
                    base=128 * c, pattern=[[1, 128]], channel_multiplier=-81)
                nc.gpsimd.affine_select(
                    out=SELF0[:], in_=SELF0[:],
                    compare_op=mybir.AluOpType.is_ge, fill=0.0,
                    base=80 - 128 * c, pattern=[[-1, 128]], channel_multiplier=81)
                nc.scalar.activation(out=SELF_[:, c, :], in_=SELF0[:],
                                     func=AF.Identity)
            selp_cm.__exit__(None, None, None)

            # x lands per-macro so macro 0's features start after ~300KB,
            # not after the whole 2.25MB image
            x_sb = consts.tile([128, n_sub_total * 18], F32, tag="x_sb", name="x_sb")
            for mt in range(n_macro):
                c0, c1 = mt * n_sub * 18, (mt + 1) * n_sub * 18
                nc.sync.dma_start(out=x_sb[:, c0:c1], in_=x_d[:, c0:c1])

            W1 = {}; W2 = {}; W3 = {}; B1 = {}; B2 = {}; B3 = {}; WF = {}
            scal4 = consts.tile([1, 4], F32, tag="scal4", name="scal4")
            B3["a"] = scal4[0:1, 0:1]; B3["b"] = scal4[0:1, 1:2]
            WF["a"] = scal4[0:1, 2:3]; WF["b"] = scal4[0:1, 3:4]
            for t in ("a", "b"):
                W1[t] = consts.tile([128, 6, H], F8, tag="W1" + t, name="W1" + t)
                W2[t] = consts.tile([128, 8, H], F8, tag="W2" + t, name="W2" + t)
                W3[t] = consts.tile([128, 8, 1], BF16, tag="W3" + t, name="W3" + t)
                B1[t] = consts.tile([128, 8], F32, tag="b1" + t, name="b1" + t)
                B2[t] = consts.tile([128, 8], F32, tag="b2" + t, name="b2" + t)

            # One DMA per weight tile (region-aligned so each consumer
            # matmul waits on exactly one queue); all tensors start
            # streaming concurrently on different queues; issue order
            # matches first-use order in the tower loop.
            for t in ("a", "b"):
                nc.sync.dma_start(out=W1[t][:],
                                  in_=wd["W1" + t][:, :].rearrange("(k p) h -> p k h", p=128))
                # W2 in halves so macro 0's L2 starts on the first half
                nc.sync.dma_start(out=W2[t][:, 0:4, :],
                                  in_=wd["W2" + t][0:512, :].rearrange("(k p) h -> p k h", p=128))
                nc.sync.dma_start(out=W2[t][:, 4:8, :],
                                  in_=wd["W2" + t][512:1024, :].rearrange("(k p) h -> p k h", p=128))
            for t in ("a", "b"):
                nc.sync.dma_start(out=W3[t][:],
                                  in_=wd["W3" + t][:, :].rearrange("(k p) h -> p k h", p=128))
                nc.sync.dma_start(out=B1[t][:], in_=wd["b1" + t].rearrange("(m p) -> p m", p=128))
                nc.sync.dma_start(out=B2[t][:], in_=wd["b2" + t].rearrange("(m p) -> p m", p=128))
                nc.sync.dma_start(out=B3[t], in_=wd["b3" + t].rearrange("(p o) -> p o", o=1))
                nc.sync.dma_start(out=WF[t], in_=wd["Wf" + t][:, :])

            def feats_subtile(x_t, eng):
                """Compute the level-2/factor-2 vectors for one 128-sample
                subtile. x_t: [128, 18] slice (batch on partitions).
                eng: nc.vector or nc.gpsimd -- whole chain runs there
                (except reciprocal), so alternating subtiles pipeline on
                two engines."""
                xr = x_t[:, 0:9]
                xi = x_t[:, 9:18]

                # full 3x3 grids for all 3 factors: [128, 27], idx f*9+a*3+b
                XX = ft.tile([128, 27], F32, tag="XX", name="XX")
                eng.tensor_mul(XX[:], _brd(xr, [[3, 3], [1, 3], [0, 3]], bass),
                               _brd(xr, [[3, 3], [0, 3], [1, 3]], bass))
                XXYY = ft.tile([128, 27], F32, tag="XXYY", name="XXYY")
                eng.tensor_mul(XXYY[:], _brd(xi, [[3, 3], [1, 3], [0, 3]], bass),
                               _brd(xi, [[3, 3], [0, 3], [1, 3]], bass))
                eng.tensor_add(XXYY[:], XXYY[:], XX[:])
                # kappa [128, 3] = diag sums; kprod, inv (recip early: it
                # is DVE-only, so GpSimd subtiles need the round trip)
                kap = ft.tile([128, 3], F32, tag="kap", name="kap")
                eng.tensor_add(kap[:], _brd(XXYY[:, 0:1], [[9, 3]], bass),
                               _brd(XXYY[:, 4:5], [[9, 3]], bass))
                eng.tensor_add(kap[:], kap[:], _brd(XXYY[:, 8:9], [[9, 3]], bass))
                kp = ft.tile([128, 1], F32, tag="kp", name="kp")
                eng.tensor_mul(kp[:], kap[:, 0:1], kap[:, 1:2])
                eng.tensor_mul(kp[:], kp[:], kap[:, 2:3])
                inv = ft.tile([128, 1], F32, tag="inv", name="inv")
                nc.vector.reciprocal(inv[:], kp[:])
                XY = ft.tile([128, 27], F32, tag="XY", name="XY")
                eng.tensor_mul(XY[:], _brd(xr, [[3, 3], [1, 3], [0, 3]], bass),
                               _brd(xi, [[3, 3], [0, 3], [1, 3]], bass))

                # r_all [128, 3, 6]: triu-gather cols {0,1,2,4,5,8} of each grid
                r_all = ft.tile([128, 3, 6], F32, tag="r_all", name="r_all")
                eng.tensor_copy(_brd(r_all[:, 0, 0:3], [[6, 3], [1, 3]], bass),
                                _brd(XXYY[:, 0:3], [[9, 3], [1, 3]], bass))
                eng.tensor_copy(_brd(r_all[:, 0, 3:5], [[6, 3], [1, 2]], bass),
                                _brd(XXYY[:, 4:6], [[9, 3], [1, 2]], bass))
                eng.tensor_copy(_brd(r_all[:, 0, 5:6], [[6, 3], [1, 1]], bass),
                                _brd(XXYY[:, 8:9], [[9, 3], [1, 1]], bass))
                # im_all [128, 3, 3]: XY[a,b]-XY[b,a] for (0,1),(0,2),(1,2)
                # (im1's sign for the L2 blocks is folded into W1 host-side)
                im_all = ft.tile([128, 3, 3], F32, tag="im_all", name="im_all")
                eng.tensor_sub(_brd(im_all[:, 0, 0:2], [[3, 3], [1, 2]], bass),
                               _brd(XY[:, 1:3], [[9, 3], [1, 2]], bass),
                               _brd(XY[:, 3:7], [[9, 3], [3, 2]], bass))
                eng.tensor_sub(_brd(im_all[:, 0, 2:3], [[3, 3], [1, 1]], bass),
                               _brd(XY[:, 5:6], [[9, 3], [1, 1]], bass),
                               _brd(XY[:, 7:8], [[9, 3], [1, 1]], bass))
                r0 = r_all[:, 0, :]
                r1 = r_all[:, 1, :]
                i0 = im_all[:, 0, :]
                i1 = im_all[:, 1, :]
                # LF [128, 90]: level-2 vector L2 (81 = [R2 45 | I2 36]) and
                # kappa-folded factor-2 vector f2 (9 = [rr2 6 | ii2 3]),
                # batch on partitions. One PE transpose turns it into
                # [90, 128] feature-on-partition.
                LF = ft.tile([128, 90], BF16, tag="LF", name="LF")
                eng.tensor_mul(LF[:, 0:36], _brd(r0, [[1, 6], [0, 6]], bass),
                               _brd(r1, [[0, 6], [1, 6]], bass))
                eng.tensor_mul(LF[:, 36:45], _brd(i0, [[1, 3], [0, 3]], bass),
                               _brd(i1[:, 0:3], [[0, 3], [1, 3]], bass))
                eng.tensor_mul(LF[:, 45:63], _brd(r0, [[1, 6], [0, 3]], bass),
                               _brd(i1[:, 0:3], [[0, 6], [1, 3]], bass))
                eng.tensor_mul(LF[:, 63:81], _brd(i0, [[1, 3], [0, 6]], bass),
                               _brd(r1, [[0, 3], [1, 6]], bass))
                eng.tensor_mul(LF[:, 81:87], r_all[:, 2, :],
                               _brd(inv[:, 0:1], [[0, 6]], bass))
                eng.tensor_mul(LF[:, 87:90], im_all[:, 2, :],
                               _brd(inv[:, 0:1], [[0, 3]], bass))
                return LF

            def feats_T(mt):
                """DVE features + PE transposes/selection-matmuls ->
                featsT [128, 6, n_tile] (fp8e4, feature-on-partition,
                pre-scaled by S_F)."""
                featsT = ftp.tile([128, 6, n_tile], F8, tag="featsT", name="featsT")
                # zero the 768-pad rows of chunk 5 (DoubleRow partner of 4);
                # engines need a 32-aligned base partition, so clear 64:128
                # and let the chunk-5 mul below rewrite 64:89 (WAW-ordered)
                nc.gpsimd.memset(featsT[64:128, 5, :], 0.0)
                L2T = lftp.tile([81, n_tile], BF16, tag="L2T", name="L2T")
                f2T9 = lftp.tile([9, n_tile], BF16, tag="f2T9", name="f2T9")
                for s in range(n_sub):
                    gs = mt * n_sub + s
                    LF = feats_subtile(x_sb[:, gs * 18:gs * 18 + 18], nc.vector)
                    pT = psT.tile([128, 256], BF16, tag="pT", name="pT")
                    nc.tensor.transpose(pT[0:81, 0:128], LF[:, 0:81], ident[:])
                    nc.tensor.transpose(pT[0:9, 128:256], LF[:, 81:90], ident[:])
                    nc.vector.tensor_copy(L2T[:, s * 128:(s + 1) * 128], pT[0:81, 0:128])
                    nc.vector.tensor_copy(f2T9[:, s * 128:(s + 1) * 128], pT[0:9, 128:256])
                for c in range(6):
                    kk = SEC_CHUNKS[c]
                    psL2 = self2.tile([128, n_tile], F32, tag="psL2", name="psL2")
                    nc.tensor.matmul(psL2[:], SELL[:, c, :], L2T[:],
                                     start=True, stop=True)
                    psF2 = self2.tile([128, n_tile], F32, tag="psF2", name="psF2")
                    nc.tensor.matmul(psF2[:], SELF_[:, c, :], f2T9[:],
                                     start=True, stop=True)
                    f2S = lftp.tile([128, n_tile], F32, tag="f2S", name="f2S",
                                    bufs=1)
                    nc.vector.tensor_copy(f2S[:kk, :], psF2[:kk, :])
                    nc.vector.tensor_mul(featsT[:kk, c, :], psL2[:kk, :], f2S[:kk, :])
                return featsT

            def layer1(featsT, t):
                """L1 for one tower: fp8 DoubleRow over 3 K=256 chunk pairs."""
                q1 = qp.tile([128, 8, n_tile], F8, tag="q1" + t, name="q1" + t,
                             bufs=2)
                for m in range(8):
                    ps = psL.tile([128, n_tile], F32, tag="mm", name="mm")
                    for c in range(3):
                        nc.tensor.matmul(ps[:],
                                         W1[t][:, 2 * c:2 * c + 2, m * 128:(m + 1) * 128],
                                         featsT[:, 2 * c:2 * c + 2, :],
                                         start=(c == 0), stop=(c == 2),
                                         perf_mode=DR)
                    nc.scalar.activation(out=q1[:, m, :], in_=ps[:], func=AF.Square,
                                         bias=B1[t][:, m:m + 1], scale=ACT1_SCALE)
                return q1

            def layer23(mt, q1s):
                """L2 (fp8 DR) with fused bf16 L3 accumulation + epilogue."""
                base = mt * n_tile
                lns = {}
                for t in ("a", "b"):
                    q1 = q1s[t]
                    u3 = psU.tile([1, n_tile], F32, tag="u3", name="u3")
                    q2_pend = []
                    for m in range(8):
                        ps = psL.tile([128, n_tile], F32, tag="mm", name="mm")
                        for c in range(4):
                            nc.tensor.matmul(ps[:],
                                             W2[t][:, 2 * c:2 * c + 2, m * 128:(m + 1) * 128],
                                             q1[:, 2 * c:2 * c + 2, :],
                                             start=(c == 0), stop=(c == 3),
                                             perf_mode=DR)
                        q2m = q2p.tile([128, n_tile], BF16, tag="q2m", name="q2m")
                        nc.scalar.activation(out=q2m[:], in_=ps[:], func=AF.Square,
                                             bias=B2[t][:, m:m + 1], scale=ACT2S[t])
                        # lag the u3 accumulation one m-chunk so the in-order
                        # PE never waits on the ACT square it just requested
                        q2_pend.append((m, q2m))
                        if len(q2_pend) == 2:
                            mm_, q2m_ = q2_pend.pop(0)
                            nc.tensor.matmul(u3[:], W3[t][:, mm_, :], q2m_[:],
                                             start=(mm_ == 0), stop=False)
                    mm_, q2m_ = q2_pend.pop(0)
                    nc.tensor.matmul(u3[:], W3[t][:, mm_, :], q2m_[:],
                                     start=False, stop=True)
                    ln_t = ep.tile([1, n_tile], F32, tag="ln" + t, name="ln" + t)
                    nc.scalar.activation(out=ln_t[:], in_=u3[:], func=AF.Square,
                                         bias=B3[t], scale=1.0)
                    nc.scalar.activation(out=ln_t[:], in_=ln_t[:], func=AF.Ln)
                    lns[t] = ln_t
                nc.vector.tensor_scalar_mul(lns["b"][:], lns["b"][:], WF["b"])
                nc.vector.scalar_tensor_tensor(out=lns["a"][:], in0=lns["a"][:],
                                               scalar=WF["a"], in1=lns["b"][:],
                                               op0=ALU.mult, op1=ALU.subtract)
                nc.vector.tensor_scalar(out=lns["a"][:], in0=lns["a"][:], scalar1=1.0e6,
                                        scalar2=-1.0e6, op0=ALU.min, op1=ALU.max)
                nc.sync.dma_start(out=out_d[base:base + n_tile], in_=lns["a"][:])

            # Software-pipelined emission: features + tower-a L1 run one
            # macro ahead (they only need x and W1a, so they fill the PE
            # while the W2/W3 DMAs land); tower-b L1 is emitted just before
            # its consumer to keep q1b at one buffer.
            fT = {0: feats_T(0)}
            q1a_pre = {}
            for mt in range(n_macro):
                if mt + 1 < n_macro:
                    fT[mt + 1] = feats_T(mt + 1)
                    if mt == 0:
                        # startup-only: macro 1's tower-a L1 needs just W1a,
                        # giving the PE work while the W2/W3 DMAs land
                        q1a_pre[1] = layer1(fT[1], "a")
                cur = fT.pop(mt)
                q1a = q1a_pre.pop(mt) if mt in q1a_pre else layer1(cur, "a")
                q1b = layer1(cur, "b")
                layer23(mt, {"a": q1a, "b": q1b})

    if finalize:
        nc.finalize()   # Bacc pass pipeline: reg alloc, wait splitting, etc.
    return nc


def _w1_perm():
    """featsT row g = j*81 + i holds L2vec[i]*f2vec[j]; reference sec index
    for that product (blocks: R2xrr2 | I2nxii2 | R2xii2 | I2nxrr2)."""
    perm = np.empty(NSEC, np.int64)
    for j in range(9):
        for i in range(81):
            if i < 45:
                ref = i * 6 + j if j < 6 else 378 + i * 3 + (j - 6)
            else:
                ii = i - 45
                ref = 513 + ii * 6 + j if j < 6 else 270 + ii * 3 + (j - 6)
            perm[j * 81 + i] = ref
    return perm


def _to_e4m3(w, scale):
    """Scale, clip to TRN e4m3 range, quantize (RNE)."""
    import ml_dtypes
    return np.clip(w * scale, -240.0, 240.0).astype(ml_dtypes.float8_e4m3)


def prep_weights(inputs):
    """Per-core weight dict + per-tower L2 ACT scale.

    W1: rows permuted to the kernel's featsT order, padded to 768, scaled
    e4m3. W2: scaled e4m3 (per-tower pow2 scale). W3: bf16. b1 pre-scaled
    by sqrt(S_Q1) to match the L1 ACT square."""
    import ml_dtypes
    inp = {k: np.asarray(v, dtype=np.float32) for k, v in inputs.items()
           if k != "x"}
    perm = _w1_perm()
    weights = {}
    act2s = {}
    # the kernel computes L2[36:63] with +im1 (no negation op); the
    # reference wants -im1 there, so those W1 rows flip sign
    i_idx = np.arange(768) % 81
    sgn = np.where((i_idx >= 36) & (i_idx < 63), -1.0, 1.0).astype(np.float32)
    for t in ("a", "b"):
        w1 = np.zeros((768, H), np.float32)
        w1[:NSEC] = inp["W1" + t][perm]
        w1 *= sgn[:, None]
        weights["W1" + t] = np.ascontiguousarray(_to_e4m3(w1, S_W1))
        weights["b1" + t] = (inp["b1" + t] * np.sqrt(S_Q1)).astype(np.float32)
        w2 = inp["W2" + t]
        s_w2 = float(2.0 ** np.floor(np.log2(96.0 / max(np.abs(w2).max(), 1e-30))))
        weights["W2" + t] = np.ascontiguousarray(_to_e4m3(w2, s_w2))
        act2s[t] = 1.0 / (S_Q1 * s_w2)
        weights["b2" + t] = inp["b2" + t]
        weights["W3" + t] = inp["W3" + t].astype(ml_dtypes.bfloat16)
        weights["b3" + t] = inp["b3" + t]
        weights["Wf" + t] = inp["Wf" + t]
    return weights, act2s


def prep_x(x_core):
    """Per-core x [b, 18] -> SBUF image [128, (b/128)*18]: partition p holds
    sample p of each 128-row subtile, subtiles concatenated along free dim."""
    b = x_core.shape[0]
    n_sub_total = b // 128
    return np.ascontiguousarray(
        x_core.reshape(n_sub_total, 128, 18).transpose(1, 0, 2).reshape(128, n_sub_total * 18))


def run(inputs, trace=False, b_core=B_CORE, n_tile=N_TILE, n_cores=N_CORES):
    """Shard inputs, run the SPMD kernel on n_cores, gather full output.
    Returns (out [B,1] fp32, BassKernelResults)."""
    from concourse import bass_utils

    weights, act2s = prep_weights(inputs)
    nc = build_nc(act2s, b_core=b_core, n_tile=n_tile)
    x = np.ascontiguousarray(np.asarray(inputs["x"], dtype=np.float32))
    in_maps = []
    for c in range(n_cores):
        m = {"x": prep_x(x[c * b_core:(c + 1) * b_core])}
        m.update(weights)
        in_maps.append(m)
    res = bass_utils.run_bass_kernel_spmd(nc, in_maps, core_ids=list(range(n_cores)),
                                          trace=trace)
    out = np.concatenate([r["out"] for r in res.results], axis=0)
    return out.reshape(-1, 1).astype(np.float32), res


def kernel(**inputs) -> np.ndarray:
    out, _ = run(inputs, trace=False)
    return out
